# revision 27
# baseline (speedup 1.0000x reference)
"""Falcon-style MQA attention (71 heads, 1 KV head, RoPE, causal) on 8 TRN2 NeuronCores.

Sharding: tensor-parallel over query heads (9 per core, core 7 has 8 + a zero-pad
head), the single KV head replicated. Per core: QKV projection for its heads
(+KV), RoPE, causal flash-style attention in transposed layout, then a PARTIAL
dense projection over the core's own head rows for all 4544 output columns.
The host sums the 8 partial outputs (contraction-sharded dense = host reduce);
no device collective is needed. All operand transposes are done host-side.

Schedule (v2): the kernel is software-pipelined across the two batches so the
tensor engine always has dense matmul work (keeps the PE HAM clock warm):
  QKV(b0) -> [attn(b0) || QKV(b1)+RoPE(b1)+prep(b1)] -> [attn(b1) || dense(b0)]
  -> dense(b1)
Interleaving is done at emission time via generators (the Tile scheduler
executes per-engine streams roughly in program order).

Self-contained: hardcodes all shapes; needs only numpy + ml_dtypes + concourse.
"""

import math
from collections import deque
from contextlib import ExitStack

import numpy as np
import ml_dtypes

import concourse.bass as bass
import concourse.mybir as mybir
import concourse.tile as tile
from concourse import bacc
from concourse.bass_utils import run_bass_kernel_spmd

NCORES = 8
N, L, D = 2, 1024, 4544
H, DKV = 71, 64
M = N * L                    # 2048 tokens
DP = 4608                    # D padded to 36*128
KT = DP // 128               # 36 contraction tiles for QKV
HPC = 9                      # head slots per core (core 7: 8 real + 1 zero-pad)
QROWS = HPC * DKV            # 576 attention rows per core
QPAD = 640                   # padded to 5*128 for the dense contraction
RROWS = QROWS + 2 * DKV      # 704 fused rows per core (q + k + v)
RC = 6                       # row-chunks of fusedT (5x128 + 64)
MCH = 256                    # QKV token-chunk width
NMC = M // MCH               # 8 token chunks (0-3 batch 0, 4-7 batch 1)
ROPE_BASE = 10000.0

F32 = mybir.dt.float32
F32R = mybir.dt.float32r
BF16 = mybir.dt.bfloat16


def _build():
    nc = bacc.Bacc("TRN2", target_bir_lowering=False, debug=False, num_devices=NCORES)

    hs_bf = nc.dram_tensor("hs_bf", [DP, M], BF16, kind="ExternalInput")      # hs.T
    wq_bf = nc.dram_tensor("wq_bf", [DP, RROWS], BF16, kind="ExternalInput")  # wq_loc.T
    wd_bf = nc.dram_tensor("wd_bf", [QPAD, D], BF16, kind="ExternalInput")    # wd rows for local heads
    cos2 = nc.dram_tensor("cos2", [128, L], BF16, kind="ExternalInput")
    sin2 = nc.dram_tensor("sin2", [128, L], BF16, kind="ExternalInput")
    tri_in = nc.dram_tensor("tri", [128, 128], BF16, kind="ExternalInput")
    prope2 = nc.dram_tensor("prope2", [128, 128], BF16, kind="ExternalInput")
    ident64 = nc.dram_tensor("ident64", [64, 64], BF16, kind="ExternalInput")
    colones = nc.dram_tensor("colones", [128, 1024], BF16, kind="ExternalInput")
    out = nc.dram_tensor("out", [M, D], BF16, kind="ExternalOutput")

    with tile.TileContext(nc) as tc, ExitStack() as top:
        constp = top.enter_context(tc.tile_pool(name="const", bufs=1))
        workp = top.enter_context(tc.tile_pool(name="work", bufs=3))
        expp = top.enter_context(tc.tile_pool(name="exps", bufs=6))
        psQ = top.enter_context(tc.tile_pool(name="psQ", bufs=2, space="PSUM"))
        psS = top.enter_context(tc.tile_pool(name="psS", bufs=3, space="PSUM"))
        psAV = top.enter_context(tc.tile_pool(name="psAV", bufs=2, space="PSUM"))
        psM = top.enter_context(tc.tile_pool(name="psM", bufs=1, space="PSUM"))

        fusedp = top.enter_context(tc.tile_pool(name="fused", bufs=1))
        fusedT = fusedp.tile([128, RC, M], BF16)

        attnp = top.enter_context(tc.tile_pool(name="attn", bufs=1))
        attn_sb = attnp.tile([128, QPAD // 128, M], BF16)
        kT_dup = attnp.tile([128, N, L], BF16)
        # v_nat columns 0-63: v.T per key block; columns 64-127: all-ones, so
        # the AV matmul's output rows 64-127 are 64 broadcast copies of the
        # softmax denominator (enables a 64-wide parallel reciprocal).
        v_nat = attnp.tile([128, N * 8, 2 * DKV], BF16)

        stageA = ExitStack()
        wqp = stageA.enter_context(tc.tile_pool(name="wq", bufs=1))
        hstp = stageA.enter_context(tc.tile_pool(name="hst", bufs=2))

        hs_r = hs_bf[:].rearrange("(kt p) m -> p kt m", p=128)

        # Startup DMA priority: the kt-outer first chunk consumes wqT kt-tiles
        # at ~0.64us each, so wqT gets all three DMA queues (interleaved with
        # the first hs half-chunk); constants and prefetches queue after.
        hsT0 = hstp.tile([128, KT, MCH], BF16, tag="hsT")
        nc.sync.dma_start(hsT0[:, 0:6, :], hs_r[:, 0:6, 0:MCH])
        wqT = wqp.tile([128, KT, RROWS], BF16)
        wq_r = wq_bf[:].rearrange("(kt p) r -> p kt r", p=128)
        for kg, eng in enumerate((nc.gpsimd, nc.scalar, nc.sync,
                                  nc.gpsimd, nc.scalar, nc.sync)):
            eng.dma_start(wqT[:, 6 * kg:6 * (kg + 1), :],
                          wq_r[:, 6 * kg:6 * (kg + 1), :])
        nc.sync.dma_start(hsT0[:, 6:21, :], hs_r[:, 6:21, 0:MCH])
        nc.sync.dma_start(hsT0[:, 21:KT, :], hs_r[:, 21:KT, 0:MCH])

        # ---- constants (after the hot startup DMAs) ----
        cosT = constp.tile([128, L], BF16)
        sinT = constp.tile([128, L], BF16)
        tri = constp.tile([128, 128], BF16)
        prope = constp.tile([128, 128], BF16)
        id64 = constp.tile([64, 64], BF16)
        nc.gpsimd.dma_start(cosT[:], cos2[:])
        nc.gpsimd.dma_start(sinT[:], sin2[:])
        nc.gpsimd.dma_start(tri[:], tri_in[:])
        nc.gpsimd.dma_start(prope[:], prope2[:])
        nc.gpsimd.dma_start(id64[:], ident64[:])
        nc.vector.memset(attn_sb[64:128, 4, :], 0.0)
        nc.gpsimd.dma_start(v_nat[:, :, DKV:2 * DKV],
                            colones[:].rearrange("p (s o) -> p s o", o=DKV))

        def qkv_chunk(mc, hsT=None):
            """Generator: QKV projection for one token chunk of MCH tokens."""
            if hsT is None:
                hsT = hstp.tile([128, KT, MCH], BF16, tag="hsT")
                nc.sync.dma_start(hsT[:], hs_r[:, :, MCH * mc:MCH * (mc + 1)])
            yield
            for rc in range(RC):
                rp = 128 if rc < 5 else 64
                ps = psQ.tile([128, 512], F32, tag="acc")
                for kt in range(KT):
                    nc.tensor.matmul(
                        ps[:rp, :MCH], wqT[:, kt, 128 * rc:128 * rc + rp],
                        hsT[:, kt, :], start=(kt == 0), stop=(kt == KT - 1))
                    if kt % 12 == 11:
                        yield
                nc.vector.tensor_copy(
                    fusedT[:rp, rc, MCH * mc:MCH * (mc + 1)], ps[:rp, :MCH])
                yield

        def rope_units(n, hfs=(0, 1)):
            """Generator: RoPE in place on q rows and the k row of batch n."""
            for hf in hfs:
                sl = slice(512 * hf, 512 * (hf + 1))
                for rc in range(5):
                    x = fusedT[:, rc, L * n:L * (n + 1)]
                    pp = psM.tile([128, 512], F32, tag="misc")
                    nc.tensor.matmul(pp[:], prope[:], x[:, sl],
                                     start=True, stop=True)
                    a = workp.tile([128, 512], BF16, tag="ropea")
                    b = workp.tile([128, 512], BF16, tag="ropeb")
                    nc.vector.tensor_mul(a[:], x[:, sl], cosT[:, sl])
                    nc.vector.tensor_mul(b[:], pp[:], sinT[:, sl])
                    nc.vector.tensor_add(x[:, sl], a[:], b[:])
                    yield

        def prep_vt(n, jts):
            """Generator: v transposes for batch n, given 128-key blocks."""
            for jt in jts:
                tp = psM.tile([128, 1024], BF16, tag="misc")
                nc.tensor.transpose(
                    tp[:, 0:64], fusedT[0:64, 5, L * n + 128 * jt:L * n + 128 * (jt + 1)],
                    id64[:])
                nc.vector.tensor_copy(v_nat[:, 8 * n + jt, 0:DKV], tp[:, 0:64])
                yield

        def prep_kdup(n):
            """Generator: kT duplication into both partition halves so
            lhsT/rhs base partitions match for every head parity."""
            nc.gpsimd.dma_start(kT_dup[0:64, n, :], fusedT[64:128, 4, L * n:L * (n + 1)])
            nc.gpsimd.dma_start(kT_dup[64:128, n, :], fusedT[64:128, 4, L * n:L * (n + 1)])
            yield

        def attn_head(n, h):
            """Generator: one attention head, yielding between j-tile units."""
            poff = (64 * h) % 128
            prc = (64 * h) // 128
            kTn = kT_dup[poff:poff + 64, n, :]
            qh = fusedT[poff:poff + 64, prc, L * n:L * (n + 1)]
            for qc in range(2):
                av = psAV.tile([128, 512], F32, tag="av")
                njt = 4 * (qc + 1)
                pend = None
                for jt in range(njt):
                    off = max(0, 128 * jt - 512 * qc)
                    sp = psS.tile([128, 512], F32, tag="sp")
                    nc.tensor.matmul(
                        sp[:, 0:512 - off],
                        kTn[:, 128 * jt:128 * (jt + 1)],
                        qh[:, 512 * qc + off:512 * (qc + 1)],
                        start=True, stop=True)
                    et = expp.tile([128, 512], BF16, tag="exp")
                    nc.scalar.activation(
                        et[:, off:512], sp[:, 0:512 - off],
                        mybir.ActivationFunctionType.Exp,
                        scale=1.0 / math.sqrt(DKV))
                    if 128 * jt >= 512 * qc:
                        nc.vector.tensor_mul(
                            et[:, off:off + 128], et[:, off:off + 128], tri[:])
                    if pend is not None:
                        pjt, po, pet = pend
                        nc.tensor.matmul(
                            av[:, po:512], v_nat[:, 8 * n + pjt, :], pet[:, po:512],
                            start=(pjt == 0), stop=False)
                    pend = (jt, off, et)
                    yield
                pjt, po, pet = pend
                nc.tensor.matmul(
                    av[:, po:512], v_nat[:, 8 * n + pjt, :], pet[:, po:512],
                    start=(pjt == 0), stop=True)
                # reciprocal_approx_fast mis-executes on base-shifted inputs
                # (HW-verified), so realign the denominator rows to partition 0
                # with a copy first.
                den = workp.tile([64, 512], F32, tag="den")
                nc.vector.tensor_copy(den[:], av[64:128, :])
                rec = workp.tile([64, 512], F32, tag="rec")
                nc.vector.reciprocal_approx_fast(rec[:], den[:])
                yield
                nc.vector.tensor_mul(
                    attn_sb[poff:poff + 64, prc, L * n + 512 * qc:L * n + 512 * (qc + 1)],
                    av[0:64, :], rec[:])
                yield

        def drive(n, fillers, prime=0):
            """Drive all heads of batch n with 2 rolling in-flight generators,
            advancing one filler yield per attention step."""
            for _ in range(prime):
                if fillers:
                    try:
                        next(fillers[0])
                    except StopIteration:
                        fillers.popleft()
            nxt = 0
            slots = []

            def spawn():
                nonlocal nxt
                if nxt < HPC:
                    slots.append(attn_head(n, nxt))
                    nxt += 1

            spawn()
            spawn()
            while slots:
                for g in list(slots):
                    try:
                        next(g)
                    except StopIteration:
                        slots.remove(g)
                        spawn()
                if fillers:
                    f = fillers[0]
                    try:
                        next(f)
                    except StopIteration:
                        fillers.popleft()

        def drain(fillers):
            while fillers:
                f = fillers.popleft()
                for _ in f:
                    pass

        def qkv_chunk0_ktouter():
            """First token chunk with the kt loop outermost: consumes wqT
            kt-tiles at DMA-arrival rate so the cold-start stall shrinks.
            Uses one accumulator bank from each PSUM pool (nothing else is
            live yet)."""
            accs = [psQ.tile([128, 512], F32, tag="acc", name="acc0"),
                    psQ.tile([128, 512], F32, tag="acc", name="acc1"),
                    psS.tile([128, 512], F32, tag="sp", name="acc2"),
                    psS.tile([128, 512], F32, tag="sp", name="acc3"),
                    psAV.tile([128, 512], F32, tag="av", name="acc4"),
                    psAV.tile([128, 512], F32, tag="av", name="acc5")]
            for kt in range(KT):
                for rc in range(RC):
                    rp = 128 if rc < 5 else 64
                    nc.tensor.matmul(
                        accs[rc][:rp, :MCH], wqT[:, kt, 128 * rc:128 * rc + rp],
                        hsT0[:, kt, :], start=(kt == 0), stop=(kt == KT - 1))
            for rc in range(RC):
                rp = 128 if rc < 5 else 64
                nc.vector.tensor_copy(fusedT[:rp, rc, 0:MCH], accs[rc][:rp, :MCH])

        # ---- window 1: QKV batch 0 ----
        qkv_chunk0_ktouter()
        for mc in range(1, NMC // 2):
            for _ in qkv_chunk(mc):
                pass
        # rope(b0) interleaved with the first QKV(b1) chunk to avoid a PE gap
        w1fill = deque([qkv_chunk(4)])
        for g in (rope_units(0), prep_vt(0, range(8)), prep_kdup(0)):
            for _ in g:
                if w1fill:
                    try:
                        next(w1fill[0])
                    except StopIteration:
                        w1fill.popleft()

        # stage B (dense weights + output staging) opens mid-drive, as soon
        # as the last QKV(b1) filler has been emitted and wqT/hsT are free —
        # so wdT2 is resident the moment attention(b0) finishes.
        stageB = ExitStack()
        dref = {}

        def stage_switch():
            stageA.close()
            wdp = stageB.enter_context(tc.tile_pool(name="wd", bufs=1))
            otp = stageB.enter_context(tc.tile_pool(name="ot", bufs=2))
            wdT2 = wdp.tile([128, QPAD // 128, D], BF16)
            wd_r = wd_bf[:].rearrange("(kt p) c -> p kt c", p=128)
            for kt, eng in enumerate((nc.gpsimd, nc.scalar, nc.sync,
                                      nc.gpsimd, nc.scalar)):
                eng.dma_start(wdT2[:, kt, :], wd_r[:, kt, :])
            dref["wdT2"] = wdT2
            dref["otp"] = otp
            yield

        # ---- window 2: attention(b0) || QKV(b1) + rope(b1) + prep(b1) ----
        # rope/prep(b1) slices follow the QKV(b1) chunks they depend on, so
        # attention(b1) is unblocked as early as possible.
        fillers1 = deque(w1fill)
        fillers1.append(qkv_chunk(5))
        fillers1.append(rope_units(1, hfs=(0,)))
        fillers1.append(prep_vt(1, range(0, 4)))
        fillers1.append(qkv_chunk(6))
        fillers1.append(qkv_chunk(7))
        fillers1.append(stage_switch())
        fillers1.append(rope_units(1, hfs=(1,)))
        fillers1.append(prep_vt(1, range(4, 8)))
        fillers1.append(prep_kdup(1))
        drive(0, fillers1)
        drain(fillers1)

        CCH = [512] * 8 + [448]          # dense column chunks (sum = 4544)

        def dense_mtile(n, mt, stream_out=False):
            """Generator: partial dense for one m-tile; one big out DMA
            (or per-chunk DMAs when stream_out, to shrink the kernel tail)."""
            wdT2 = dref["wdT2"]
            ot = dref["otp"].tile([128, D], BF16, tag="ot")
            rows = slice(L * n + 128 * mt, L * n + 128 * (mt + 1))
            col = 0
            for w in CCH:
                pa = psQ.tile([128, 512], F32, tag="acc")
                for kt in range(QPAD // 128):
                    nc.tensor.matmul(
                        pa[:, :w], attn_sb[:, kt, rows],
                        wdT2[:, kt, col:col + w],
                        start=(kt == 0), stop=(kt == QPAD // 128 - 1))
                nc.vector.tensor_copy(ot[:, col:col + w], pa[:, :w])
                if stream_out:
                    nc.gpsimd.dma_start(out[rows, col:col + w], ot[:, col:col + w])
                col += w
                yield
            if not stream_out:
                nc.gpsimd.dma_start(out[rows, :], ot[:])

        # ---- window 3: attention(b1) || dense(b0) ----
        fillers2 = deque(dense_mtile(0, mt) for mt in range(8))
        drive(1, fillers2, prime=10)
        drain(fillers2)

        # ---- window 4: dense(b1) ----
        drain(deque(dense_mtile(1, mt, stream_out=(mt == 7))
                    for mt in range(8)))
        stageB.close()

    nc.compile()
    return nc


_NC_CACHE = None


def _get_nc():
    global _NC_CACHE
    if _NC_CACHE is None:
        _NC_CACHE = _build()
    return _NC_CACHE


def _host_inputs(hidden_states, w_qkv, w_dense):
    """Build the per-core input maps (transpose + slice + bf16 cast on host)."""
    hs = np.asarray(hidden_states, dtype=np.float32).reshape(M, D)
    w_qkv = np.asarray(w_qkv, dtype=np.float32)
    w_dense = np.asarray(w_dense, dtype=np.float32)
    hs_bf = np.zeros((DP, M), dtype=ml_dtypes.bfloat16)
    hs_bf[:D, :] = np.ascontiguousarray(hs.T).astype(ml_dtypes.bfloat16)

    # RoPE tables, transposed to [dkv, l], duplicated on partitions 0-63 / 64-127
    inv_freq = 1.0 / (ROPE_BASE ** (np.arange(0, DKV, 2, dtype=np.float32) / DKV))
    t = np.arange(L, dtype=np.float32)
    freqs = np.outer(t, inv_freq)
    emb = np.concatenate([freqs, freqs], axis=-1)        # [L, DKV]
    cosT = np.cos(emb).T.astype(np.float32)              # [DKV, L]
    sinT = np.sin(emb).T.astype(np.float32)
    cos2 = np.concatenate([cosT, cosT], axis=0).astype(ml_dtypes.bfloat16)
    sin2 = np.concatenate([sinT, sinT], axis=0).astype(ml_dtypes.bfloat16)

    # tri[j, q] = 1 if j <= q (within-tile causal mask)
    tri = (np.arange(128)[:, None] <= np.arange(128)[None, :]).astype(
        ml_dtypes.bfloat16)

    # RoPE rotation: (P x)[d] = -x[d+32] (d<32), x[d-32] (d>=32); lhsT = P.T, 2 blocks
    P1 = np.zeros((DKV, DKV), dtype=np.float32)
    for d in range(32):
        P1[d, d + 32] = -1.0
        P1[d + 32, d] = 1.0
    PT = P1.T
    prope2 = np.zeros((128, 128), dtype=np.float32)
    prope2[:64, :64] = PT
    prope2[64:, 64:] = PT
    prope2 = prope2.astype(ml_dtypes.bfloat16)

    ident64 = np.eye(64, dtype=np.float32).astype(ml_dtypes.bfloat16)

    kv_bf = w_qkv[H * DKV:, :].T.astype(ml_dtypes.bfloat16)   # [D, 128]
    in_maps = []
    for c in range(NCORES):
        h0 = HPC * c
        nh = min(HPC, H - h0)
        wq_loc = np.zeros((DP, RROWS), dtype=ml_dtypes.bfloat16)
        wq_loc[:D, :nh * DKV] = w_qkv[h0 * DKV:(h0 + nh) * DKV, :].T.astype(
            ml_dtypes.bfloat16)
        wq_loc[:D, QROWS:] = kv_bf

        # dense weight rows for this core's heads: w_dense columns
        # [64*h0 : 64*(h0+nh)) transposed, zero-padded to QPAD rows
        wd_loc = np.zeros((QPAD, D), dtype=ml_dtypes.bfloat16)
        wd_loc[:nh * DKV, :] = w_dense[:, DKV * h0:DKV * (h0 + nh)].T.astype(
            ml_dtypes.bfloat16)

        in_maps.append({
            "hs_bf": hs_bf,
            "wq_bf": wq_loc,
            "wd_bf": wd_loc,
            "cos2": cos2,
            "sin2": sin2,
            "tri": tri,
            "prope2": prope2,
            "ident64": ident64,
            "colones": np.ones((128, 1024), dtype=ml_dtypes.bfloat16),
        })
    return in_maps


def kernel(hidden_states, w_qkv, w_dense, _trace=False, _trace_kwargs=None):
    nc = _get_nc()
    in_maps = _host_inputs(hidden_states, w_qkv, w_dense)
    kw = {}
    if _trace:
        kw = dict(trace=True, **(_trace_kwargs or {}))
    res = run_bass_kernel_spmd(nc, in_maps, list(range(NCORES)), **kw)
    full = res.results[0]["out"].astype(np.float32)
    for c in range(1, NCORES):
        full += res.results[c]["out"].astype(np.float32)
    kernel._last_exec_time_ns = res.exec_time_ns
    return full.reshape(N, L, D).astype(np.float32)


# revision 31
# speedup vs baseline: 1.1453x; 1.1453x over previous
"""Falcon-style MQA attention (71 heads, 1 KV head, RoPE, causal) on 8 TRN2 NeuronCores.

Sharding: tensor-parallel over query heads (9 per core, core 7 has 8 + a zero-pad
head), the single KV head replicated. Per core: QKV projection for its heads
(+KV), RoPE, causal flash-style attention in transposed layout, then a PARTIAL
dense projection over the core's own head rows for all 4544 output columns.
The host sums the 8 partial outputs (contraction-sharded dense = host reduce);
no device collective is needed. All operand transposes are done host-side.

Schedule (v2): the kernel is software-pipelined across the two batches so the
tensor engine always has dense matmul work (keeps the PE HAM clock warm):
  QKV(b0) -> [attn(b0) || QKV(b1)+RoPE(b1)+prep(b1)] -> [attn(b1) || dense(b0)]
  -> dense(b1)
Interleaving is done at emission time via generators (the Tile scheduler
executes per-engine streams roughly in program order).

Self-contained: hardcodes all shapes; needs only numpy + ml_dtypes + concourse.
"""

import math
from collections import deque
from contextlib import ExitStack

import numpy as np
import ml_dtypes

import concourse.bass as bass
import concourse.mybir as mybir
import concourse.tile as tile
from concourse import bacc
from concourse.bass_utils import run_bass_kernel_spmd

NCORES = 8
N, L, D = 2, 1024, 4544
H, DKV = 71, 64
M = N * L                    # 2048 tokens
DP = 4608                    # D padded to 36*128
KT = DP // 128               # 36 contraction tiles for QKV
HPC = 9                      # head slots per core (core 7: 8 real + 1 zero-pad)
QROWS = HPC * DKV            # 576 attention rows per core
QPAD = 640                   # padded to 5*128 for the dense contraction
RROWS = QROWS + 2 * DKV      # 704 fused rows per core (q + k + v)
RC = 6                       # row-chunks of fusedT (5x128 + 64)
MCH = 256                    # QKV token-chunk width
NMC = M // MCH               # 8 token chunks (0-3 batch 0, 4-7 batch 1)
ROPE_BASE = 10000.0

F32 = mybir.dt.float32
F32R = mybir.dt.float32r
BF16 = mybir.dt.bfloat16


def _build():
    nc = bacc.Bacc("TRN2", target_bir_lowering=False, debug=False, num_devices=NCORES)

    hs_bf = nc.dram_tensor("hs_bf", [DP, M], BF16, kind="ExternalInput")      # hs.T
    wq_bf = nc.dram_tensor("wq_bf", [DP, RROWS], BF16, kind="ExternalInput")  # wq_loc.T
    wd_bf = nc.dram_tensor("wd_bf", [QPAD, D], BF16, kind="ExternalInput")    # wd rows for local heads
    cos2 = nc.dram_tensor("cos2", [128, L], BF16, kind="ExternalInput")
    sin2 = nc.dram_tensor("sin2", [128, L], BF16, kind="ExternalInput")
    tri_in = nc.dram_tensor("tri", [128, 128], BF16, kind="ExternalInput")
    prope2 = nc.dram_tensor("prope2", [128, 128], BF16, kind="ExternalInput")
    ident64 = nc.dram_tensor("ident64", [64, 64], BF16, kind="ExternalInput")
    colones = nc.dram_tensor("colones", [128, 1024], BF16, kind="ExternalInput")
    out = nc.dram_tensor("out", [M, D], BF16, kind="ExternalOutput")

    with tile.TileContext(nc) as tc, ExitStack() as top:
        constp = top.enter_context(tc.tile_pool(name="const", bufs=1))
        workp = top.enter_context(tc.tile_pool(name="work", bufs=3))
        expp = top.enter_context(tc.tile_pool(name="exps", bufs=6))
        psQ = top.enter_context(tc.tile_pool(name="psQ", bufs=2, space="PSUM"))
        psS = top.enter_context(tc.tile_pool(name="psS", bufs=2, space="PSUM"))
        psAV = top.enter_context(tc.tile_pool(name="psAV", bufs=2, space="PSUM"))
        psM = top.enter_context(tc.tile_pool(name="psM", bufs=2, space="PSUM"))

        fusedp = top.enter_context(tc.tile_pool(name="fused", bufs=1))
        fusedT = fusedp.tile([128, RC, M], BF16)

        attnp = top.enter_context(tc.tile_pool(name="attn", bufs=1))
        attn_sb = attnp.tile([128, QPAD // 128, M], BF16)
        kT_dup = attnp.tile([128, N, L], BF16)
        # v_nat columns 0-63: v.T per key block; columns 64-127: all-ones, so
        # the AV matmul's output rows 64-127 are 64 broadcast copies of the
        # softmax denominator (enables a 64-wide parallel reciprocal).
        v_nat = attnp.tile([128, N * 8, 2 * DKV], BF16)

        stageA = ExitStack()
        wqp = stageA.enter_context(tc.tile_pool(name="wq", bufs=1))
        hstp = stageA.enter_context(tc.tile_pool(name="hst", bufs=2))

        hs_r = hs_bf[:].rearrange("(kt p) m -> p kt m", p=128)

        # Startup: hs chunk 0 alone on the sync queue (the kt-outer sweep
        # needs it promptly); wqT split across the gpsimd+scalar queues.
        hsT0 = hstp.tile([128, KT, MCH], BF16, tag="hsT")
        nc.sync.dma_start(hsT0[:, 0:12, :], hs_r[:, 0:12, 0:MCH])
        wqT = wqp.tile([128, KT, RROWS], BF16)
        wq_r = wq_bf[:].rearrange("(kt p) r -> p kt r", p=128)
        for kg, eng in enumerate((nc.gpsimd, nc.scalar, nc.gpsimd,
                                  nc.scalar, nc.gpsimd, nc.scalar)):
            eng.dma_start(wqT[:, 6 * kg:6 * (kg + 1), :],
                          wq_r[:, 6 * kg:6 * (kg + 1), :])
        nc.sync.dma_start(hsT0[:, 12:KT, :], hs_r[:, 12:KT, 0:MCH])

        # ---- constants (after the hot startup DMAs) ----
        cosT = constp.tile([128, L], BF16)
        sinT = constp.tile([128, L], BF16)
        tri = constp.tile([128, 128], BF16)
        prope = constp.tile([128, 128], BF16)
        id64 = constp.tile([64, 64], BF16)
        nc.gpsimd.dma_start(cosT[:], cos2[:])
        nc.gpsimd.dma_start(sinT[:], sin2[:])
        nc.gpsimd.dma_start(tri[:], tri_in[:])
        nc.gpsimd.dma_start(prope[:], prope2[:])
        nc.gpsimd.dma_start(id64[:], ident64[:])
        nc.vector.memset(attn_sb[64:128, 4, :], 0.0)
        nc.gpsimd.dma_start(v_nat[:, :, DKV:2 * DKV],
                            colones[:].rearrange("p (s o) -> p s o", o=DKV))

        def qkv_chunk(mc, hsT=None):
            """Generator: QKV projection for one token chunk of MCH tokens."""
            if hsT is None:
                hsT = hstp.tile([128, KT, MCH], BF16, tag="hsT")
                # split each chunk across two DMA queues for parallel fill
                nc.sync.dma_start(hsT[:, 0:18, :],
                                  hs_r[:, 0:18, MCH * mc:MCH * (mc + 1)])
                nc.gpsimd.dma_start(hsT[:, 18:KT, :],
                                    hs_r[:, 18:KT, MCH * mc:MCH * (mc + 1)])
            yield
            for rc in range(RC):
                rp = 128 if rc < 5 else 64
                ps = psQ.tile([128, 512], F32, tag="acc")
                for kt in range(KT):
                    nc.tensor.matmul(
                        ps[:rp, :MCH], wqT[:, kt, 128 * rc:128 * rc + rp],
                        hsT[:, kt, :], start=(kt == 0), stop=(kt == KT - 1))
                    if kt % 12 == 11:
                        yield
                nc.vector.tensor_copy(
                    fusedT[:rp, rc, MCH * mc:MCH * (mc + 1)], ps[:rp, :MCH])
                yield

        def rope_units(n, hfs=(0, 1)):
            """Generator: RoPE in place on q rows and the k row of batch n."""
            for hf in hfs:
                sl = slice(512 * hf, 512 * (hf + 1))
                for rc in range(5):
                    x = fusedT[:, rc, L * n:L * (n + 1)]
                    pp = psM.tile([128, 512], F32, tag="misc")
                    nc.tensor.matmul(pp[:], prope[:], x[:, sl],
                                     start=True, stop=True)
                    a = workp.tile([128, 512], BF16, tag="ropea")
                    b = workp.tile([128, 512], BF16, tag="ropeb")
                    nc.vector.tensor_mul(a[:], x[:, sl], cosT[:, sl])
                    nc.vector.tensor_mul(b[:], pp[:], sinT[:, sl])
                    nc.vector.tensor_add(x[:, sl], a[:], b[:])
                    yield

        def prep_vt(n, jts):
            """Generator: v transposes for batch n, given 128-key blocks."""
            for jt in jts:
                tp = psM.tile([128, 1024], BF16, tag="misc")
                nc.tensor.transpose(
                    tp[:, 0:64], fusedT[0:64, 5, L * n + 128 * jt:L * n + 128 * (jt + 1)],
                    id64[:])
                nc.vector.tensor_copy(v_nat[:, 8 * n + jt, 0:DKV], tp[:, 0:64])
                yield

        def prep_kdup(n):
            """Generator: kT duplication into both partition halves so
            lhsT/rhs base partitions match for every head parity."""
            nc.gpsimd.dma_start(kT_dup[0:64, n, :], fusedT[64:128, 4, L * n:L * (n + 1)])
            nc.gpsimd.dma_start(kT_dup[64:128, n, :], fusedT[64:128, 4, L * n:L * (n + 1)])
            yield

        def attn_head(n, h):
            """Generator: one attention head, yielding between j-tile units."""
            poff = (64 * h) % 128
            prc = (64 * h) // 128
            kTn = kT_dup[poff:poff + 64, n, :]
            qh = fusedT[poff:poff + 64, prc, L * n:L * (n + 1)]
            for qc in range(2):
                av = psAV.tile([128, 512], F32, tag="av")
                njt = 4 * (qc + 1)
                pend = None
                for jt in range(njt):
                    off = max(0, 128 * jt - 512 * qc)
                    sp = psS.tile([128, 512], F32, tag="sp")
                    nc.tensor.matmul(
                        sp[:, 0:512 - off],
                        kTn[:, 128 * jt:128 * (jt + 1)],
                        qh[:, 512 * qc + off:512 * (qc + 1)],
                        start=True, stop=True)
                    et = expp.tile([128, 512], BF16, tag="exp")
                    nc.scalar.activation(
                        et[:, off:512], sp[:, 0:512 - off],
                        mybir.ActivationFunctionType.Exp,
                        scale=1.0 / math.sqrt(DKV))
                    if 128 * jt >= 512 * qc:
                        nc.vector.tensor_mul(
                            et[:, off:off + 128], et[:, off:off + 128], tri[:])
                    if pend is not None:
                        pjt, po, pet = pend
                        nc.tensor.matmul(
                            av[:, po:512], v_nat[:, 8 * n + pjt, :], pet[:, po:512],
                            start=(pjt == 0), stop=False)
                    pend = (jt, off, et)
                    yield
                pjt, po, pet = pend
                nc.tensor.matmul(
                    av[:, po:512], v_nat[:, 8 * n + pjt, :], pet[:, po:512],
                    start=(pjt == 0), stop=True)
                # reciprocal_approx_fast mis-executes on base-shifted inputs
                # (HW-verified), so realign the denominator rows to partition 0
                # with a copy first.
                den = workp.tile([64, 512], F32, tag="den")
                nc.vector.tensor_copy(den[:], av[64:128, :])
                rec = workp.tile([64, 512], F32, tag="rec")
                nc.vector.reciprocal_approx_fast(rec[:], den[:])
                yield
                nc.vector.tensor_mul(
                    attn_sb[poff:poff + 64, prc, L * n + 512 * qc:L * n + 512 * (qc + 1)],
                    av[0:64, :], rec[:])
                yield

        def drive(n, fillers, prime=0, ratio=1.6):
            """Drive all heads of batch n with 2 rolling in-flight generators,
            advancing ~ratio filler yields per attention step."""

            def advance():
                if fillers:
                    try:
                        next(fillers[0])
                    except StopIteration:
                        fillers.popleft()

            for _ in range(prime):
                advance()
            nxt = 0
            slots = []

            def spawn():
                nonlocal nxt
                if nxt < HPC:
                    slots.append(attn_head(n, nxt))
                    nxt += 1

            spawn()
            spawn()
            step = 0
            credit = 0.0
            while slots:
                for g in list(slots):
                    try:
                        next(g)
                    except StopIteration:
                        slots.remove(g)
                        spawn()
                step += 1
                credit += ratio
                while credit >= 1.0:
                    credit -= 1.0
                    advance()

        def drain(fillers):
            while fillers:
                f = fillers.popleft()
                for _ in f:
                    pass

        def qkv_chunk0_ktouter():
            """First token chunk with the kt loop outermost: consumes wqT
            kt-tiles at DMA-arrival rate so the cold-start stall shrinks.
            Uses one accumulator bank from each PSUM pool (nothing else is
            live yet)."""
            accs = [psQ.tile([128, 512], F32, tag="acc", name="acc0"),
                    psQ.tile([128, 512], F32, tag="acc", name="acc1"),
                    psS.tile([128, 512], F32, tag="sp", name="acc2"),
                    psS.tile([128, 512], F32, tag="sp", name="acc3"),
                    psAV.tile([128, 512], F32, tag="av", name="acc4"),
                    psAV.tile([128, 512], F32, tag="av", name="acc5")]
            for kt in range(KT):
                for rc in range(RC):
                    rp = 128 if rc < 5 else 64
                    nc.tensor.matmul(
                        accs[rc][:rp, :MCH], wqT[:, kt, 128 * rc:128 * rc + rp],
                        hsT0[:, kt, :], start=(kt == 0), stop=(kt == KT - 1))
            for rc in range(RC):
                rp = 128 if rc < 5 else 64
                nc.vector.tensor_copy(fusedT[:rp, rc, 0:MCH], accs[rc][:rp, :MCH])

        # ---- window 1: QKV batch 0 ----
        qkv_chunk0_ktouter()
        for mc in range(1, NMC // 2):
            for _ in qkv_chunk(mc):
                pass
        # rope(b0) interleaved with the first QKV(b1) chunk to avoid a PE gap
        w1fill = deque([qkv_chunk(4)])
        for g in (rope_units(0), prep_vt(0, range(8)), prep_kdup(0)):
            for _ in g:
                if w1fill:
                    try:
                        next(w1fill[0])
                    except StopIteration:
                        w1fill.popleft()

        # stage B (dense weights + output staging) opens mid-drive, as soon
        # as the last QKV(b1) filler has been emitted and wqT/hsT are free —
        # so wdT2 is resident the moment attention(b0) finishes.
        stageB = ExitStack()
        dref = {}

        def stage_switch():
            stageA.close()
            wdp = stageB.enter_context(tc.tile_pool(name="wd", bufs=1))
            otp = stageB.enter_context(tc.tile_pool(name="ot", bufs=2))
            wdT2 = wdp.tile([128, QPAD // 128, D], BF16)
            wd_r = wd_bf[:].rearrange("(kt p) c -> p kt c", p=128)
            for kt, eng in enumerate((nc.gpsimd, nc.scalar, nc.sync,
                                      nc.gpsimd, nc.scalar)):
                eng.dma_start(wdT2[:, kt, :], wd_r[:, kt, :])
            dref["wdT2"] = wdT2
            dref["otp"] = otp
            yield

        # ---- window 2: attention(b0) || QKV(b1) + rope(b1) + prep(b1) ----
        # rope/prep(b1) slices follow the QKV(b1) chunks they depend on, so
        # attention(b1) is unblocked as early as possible.
        fillers1 = deque(w1fill)
        fillers1.append(qkv_chunk(5))
        fillers1.append(rope_units(1, hfs=(0,)))
        fillers1.append(prep_vt(1, range(0, 4)))
        fillers1.append(qkv_chunk(6))
        fillers1.append(qkv_chunk(7))
        fillers1.append(stage_switch())
        fillers1.append(rope_units(1, hfs=(1,)))
        fillers1.append(prep_vt(1, range(4, 8)))
        fillers1.append(prep_kdup(1))
        drive(0, fillers1)
        drain(fillers1)

        CCH = [512] * 8 + [448]          # dense column chunks (sum = 4544)

        def dense_mtile(n, mt, stream_out=False):
            """Generator: partial dense for one m-tile; one big out DMA
            (or per-chunk DMAs when stream_out, to shrink the kernel tail)."""
            wdT2 = dref["wdT2"]
            ot = dref["otp"].tile([128, D], BF16, tag="ot")
            rows = slice(L * n + 128 * mt, L * n + 128 * (mt + 1))
            col = 0
            for w in CCH:
                pa = psQ.tile([128, 512], F32, tag="acc")
                for kt in range(QPAD // 128):
                    nc.tensor.matmul(
                        pa[:, :w], attn_sb[:, kt, rows],
                        wdT2[:, kt, col:col + w],
                        start=(kt == 0), stop=(kt == QPAD // 128 - 1))
                nc.vector.tensor_copy(ot[:, col:col + w], pa[:, :w])
                if stream_out:
                    nc.gpsimd.dma_start(out[rows, col:col + w], ot[:, col:col + w])
                col += w
                yield
            if not stream_out:
                nc.gpsimd.dma_start(out[rows, :], ot[:])

        # ---- window 3: attention(b1) || dense(b0) ----
        fillers2 = deque(dense_mtile(0, mt) for mt in range(8))
        drive(1, fillers2, prime=10)
        drain(fillers2)

        # ---- window 4: dense(b1) ----
        drain(deque(dense_mtile(1, mt, stream_out=(mt == 7))
                    for mt in range(8)))
        stageB.close()

    nc.compile()
    return nc


_NC_CACHE = None


def _get_nc():
    global _NC_CACHE
    if _NC_CACHE is None:
        _NC_CACHE = _build()
    return _NC_CACHE


def _host_inputs(hidden_states, w_qkv, w_dense):
    """Build the per-core input maps (transpose + slice + bf16 cast on host)."""
    hs = np.asarray(hidden_states, dtype=np.float32).reshape(M, D)
    w_qkv = np.asarray(w_qkv, dtype=np.float32)
    w_dense = np.asarray(w_dense, dtype=np.float32)
    hs_bf = np.zeros((DP, M), dtype=ml_dtypes.bfloat16)
    hs_bf[:D, :] = np.ascontiguousarray(hs.T).astype(ml_dtypes.bfloat16)

    # RoPE tables, transposed to [dkv, l], duplicated on partitions 0-63 / 64-127
    inv_freq = 1.0 / (ROPE_BASE ** (np.arange(0, DKV, 2, dtype=np.float32) / DKV))
    t = np.arange(L, dtype=np.float32)
    freqs = np.outer(t, inv_freq)
    emb = np.concatenate([freqs, freqs], axis=-1)        # [L, DKV]
    cosT = np.cos(emb).T.astype(np.float32)              # [DKV, L]
    sinT = np.sin(emb).T.astype(np.float32)
    cos2 = np.concatenate([cosT, cosT], axis=0).astype(ml_dtypes.bfloat16)
    sin2 = np.concatenate([sinT, sinT], axis=0).astype(ml_dtypes.bfloat16)

    # tri[j, q] = 1 if j <= q (within-tile causal mask)
    tri = (np.arange(128)[:, None] <= np.arange(128)[None, :]).astype(
        ml_dtypes.bfloat16)

    # RoPE rotation: (P x)[d] = -x[d+32] (d<32), x[d-32] (d>=32); lhsT = P.T, 2 blocks
    P1 = np.zeros((DKV, DKV), dtype=np.float32)
    for d in range(32):
        P1[d, d + 32] = -1.0
        P1[d + 32, d] = 1.0
    PT = P1.T
    prope2 = np.zeros((128, 128), dtype=np.float32)
    prope2[:64, :64] = PT
    prope2[64:, 64:] = PT
    prope2 = prope2.astype(ml_dtypes.bfloat16)

    ident64 = np.eye(64, dtype=np.float32).astype(ml_dtypes.bfloat16)

    kv_bf = w_qkv[H * DKV:, :].T.astype(ml_dtypes.bfloat16)   # [D, 128]
    in_maps = []
    for c in range(NCORES):
        h0 = HPC * c
        nh = min(HPC, H - h0)
        wq_loc = np.zeros((DP, RROWS), dtype=ml_dtypes.bfloat16)
        wq_loc[:D, :nh * DKV] = w_qkv[h0 * DKV:(h0 + nh) * DKV, :].T.astype(
            ml_dtypes.bfloat16)
        wq_loc[:D, QROWS:] = kv_bf

        # dense weight rows for this core's heads: w_dense columns
        # [64*h0 : 64*(h0+nh)) transposed, zero-padded to QPAD rows
        wd_loc = np.zeros((QPAD, D), dtype=ml_dtypes.bfloat16)
        wd_loc[:nh * DKV, :] = w_dense[:, DKV * h0:DKV * (h0 + nh)].T.astype(
            ml_dtypes.bfloat16)

        in_maps.append({
            "hs_bf": hs_bf,
            "wq_bf": wq_loc,
            "wd_bf": wd_loc,
            "cos2": cos2,
            "sin2": sin2,
            "tri": tri,
            "prope2": prope2,
            "ident64": ident64,
            "colones": np.ones((128, 1024), dtype=ml_dtypes.bfloat16),
        })
    return in_maps


def kernel(hidden_states, w_qkv, w_dense, _trace=False, _trace_kwargs=None):
    nc = _get_nc()
    in_maps = _host_inputs(hidden_states, w_qkv, w_dense)
    kw = {}
    if _trace:
        kw = dict(trace=True, **(_trace_kwargs or {}))
    res = run_bass_kernel_spmd(nc, in_maps, list(range(NCORES)), **kw)
    full = res.results[0]["out"].astype(np.float32)
    for c in range(1, NCORES):
        full += res.results[c]["out"].astype(np.float32)
    kernel._last_exec_time_ns = res.exec_time_ns
    return full.reshape(N, L, D).astype(np.float32)


# revision 32
# speedup vs baseline: 1.1756x; 1.0264x over previous
"""Falcon-style MQA attention (71 heads, 1 KV head, RoPE, causal) on 8 TRN2 NeuronCores.

Sharding: tensor-parallel over query heads (9 per core, core 7 has 8 + a zero-pad
head), the single KV head replicated. Per core: QKV projection for its heads
(+KV), RoPE, causal flash-style attention in transposed layout, then a PARTIAL
dense projection over the core's own head rows for all 4544 output columns.
The host sums the 8 partial outputs (contraction-sharded dense = host reduce);
no device collective is needed. All operand transposes are done host-side.

Schedule (v2): the kernel is software-pipelined across the two batches so the
tensor engine always has dense matmul work (keeps the PE HAM clock warm):
  QKV(b0) -> [attn(b0) || QKV(b1)+RoPE(b1)+prep(b1)] -> [attn(b1) || dense(b0)]
  -> dense(b1)
Interleaving is done at emission time via generators (the Tile scheduler
executes per-engine streams roughly in program order).

Self-contained: hardcodes all shapes; needs only numpy + ml_dtypes + concourse.
"""

import math
from collections import deque
from contextlib import ExitStack

import numpy as np
import ml_dtypes

import concourse.bass as bass
import concourse.mybir as mybir
import concourse.tile as tile
from concourse import bacc
from concourse.bass_utils import run_bass_kernel_spmd

NCORES = 8
N, L, D = 2, 1024, 4544
H, DKV = 71, 64
M = N * L                    # 2048 tokens
DP = 4608                    # D padded to 36*128
KT = DP // 128               # 36 contraction tiles for QKV
HPC = 9                      # head slots per core (core 7: 8 real + 1 zero-pad)
QROWS = HPC * DKV            # 576 attention rows per core
QPAD = 640                   # padded to 5*128 for the dense contraction
RROWS = QROWS + 2 * DKV      # 704 fused rows per core (q + k + v)
RC = 6                       # row-chunks of fusedT (5x128 + 64)
MCH = 256                    # QKV token-chunk width
NMC = M // MCH               # 8 token chunks (0-3 batch 0, 4-7 batch 1)
ROPE_BASE = 10000.0

F32 = mybir.dt.float32
F32R = mybir.dt.float32r
BF16 = mybir.dt.bfloat16


def _build():
    nc = bacc.Bacc("TRN2", target_bir_lowering=False, debug=False, num_devices=NCORES)

    hs_bf = nc.dram_tensor("hs_bf", [DP, M], BF16, kind="ExternalInput")      # hs.T
    wq_bf = nc.dram_tensor("wq_bf", [DP, RROWS], BF16, kind="ExternalInput")  # wq_loc.T
    wd_bf = nc.dram_tensor("wd_bf", [QPAD, D], BF16, kind="ExternalInput")    # wd rows for local heads
    cos2 = nc.dram_tensor("cos2", [128, L], BF16, kind="ExternalInput")
    sin2 = nc.dram_tensor("sin2", [128, L], BF16, kind="ExternalInput")
    tri_in = nc.dram_tensor("tri", [128, 128], BF16, kind="ExternalInput")
    prope2 = nc.dram_tensor("prope2", [128, 128], BF16, kind="ExternalInput")
    ident64 = nc.dram_tensor("ident64", [64, 64], BF16, kind="ExternalInput")
    colones = nc.dram_tensor("colones", [128, 1024], BF16, kind="ExternalInput")
    out = nc.dram_tensor("out", [M, D], BF16, kind="ExternalOutput")

    with tile.TileContext(nc) as tc, ExitStack() as top:
        constp = top.enter_context(tc.tile_pool(name="const", bufs=1))
        workp = top.enter_context(tc.tile_pool(name="work", bufs=3))
        expp = top.enter_context(tc.tile_pool(name="exps", bufs=6))
        psQ = top.enter_context(tc.tile_pool(name="psQ", bufs=2, space="PSUM"))
        psS = top.enter_context(tc.tile_pool(name="psS", bufs=3, space="PSUM"))
        psAV = top.enter_context(tc.tile_pool(name="psAV", bufs=2, space="PSUM"))
        psM = top.enter_context(tc.tile_pool(name="psM", bufs=1, space="PSUM"))

        fusedp = top.enter_context(tc.tile_pool(name="fused", bufs=1))
        fusedT = fusedp.tile([128, RC, M], BF16)

        attnp = top.enter_context(tc.tile_pool(name="attn", bufs=1))
        attn_sb = attnp.tile([128, QPAD // 128, M], BF16)
        kT_dup = attnp.tile([128, N, L], BF16)
        # v_nat columns 0-63: v.T per key block; columns 64-127: all-ones, so
        # the AV matmul's output rows 64-127 are 64 broadcast copies of the
        # softmax denominator (enables a 64-wide parallel reciprocal).
        v_nat = attnp.tile([128, N * 8, 2 * DKV], BF16)

        stageA = ExitStack()
        wqp = stageA.enter_context(tc.tile_pool(name="wq", bufs=1))
        hstp = stageA.enter_context(tc.tile_pool(name="hst", bufs=2))

        hs_r = hs_bf[:].rearrange("(kt p) m -> p kt m", p=128)

        # Startup: hs chunk 0 alone on the sync queue (the kt-outer sweep
        # needs it promptly); wqT split across the gpsimd+scalar queues.
        hsT0 = hstp.tile([128, KT, MCH], BF16, tag="hsT")
        nc.sync.dma_start(hsT0[:, 0:12, :], hs_r[:, 0:12, 0:MCH])
        wqT = wqp.tile([128, KT, RROWS], BF16)
        wq_r = wq_bf[:].rearrange("(kt p) r -> p kt r", p=128)
        for kg, eng in enumerate((nc.gpsimd, nc.scalar, nc.gpsimd,
                                  nc.scalar, nc.gpsimd, nc.scalar)):
            eng.dma_start(wqT[:, 6 * kg:6 * (kg + 1), :],
                          wq_r[:, 6 * kg:6 * (kg + 1), :])
        nc.sync.dma_start(hsT0[:, 12:KT, :], hs_r[:, 12:KT, 0:MCH])

        # ---- constants (after the hot startup DMAs) ----
        cosT = constp.tile([128, L], BF16)
        sinT = constp.tile([128, L], BF16)
        tri = constp.tile([128, 128], BF16)
        prope = constp.tile([128, 128], BF16)
        id64 = constp.tile([64, 64], BF16)
        nc.gpsimd.dma_start(cosT[:], cos2[:])
        nc.gpsimd.dma_start(sinT[:], sin2[:])
        nc.gpsimd.dma_start(tri[:], tri_in[:])
        nc.gpsimd.dma_start(prope[:], prope2[:])
        nc.gpsimd.dma_start(id64[:], ident64[:])
        nc.vector.memset(attn_sb[64:128, 4, :], 0.0)
        nc.gpsimd.dma_start(v_nat[:, :, DKV:2 * DKV],
                            colones[:].rearrange("p (s o) -> p s o", o=DKV))

        def qkv_chunk(mc, hsT=None):
            """Generator: QKV projection for one token chunk of MCH tokens."""
            if hsT is None:
                hsT = hstp.tile([128, KT, MCH], BF16, tag="hsT")
                nc.sync.dma_start(hsT[:], hs_r[:, :, MCH * mc:MCH * (mc + 1)])
            yield
            for rc in range(RC):
                rp = 128 if rc < 5 else 64
                ps = psQ.tile([128, 512], F32, tag="acc")
                for kt in range(KT):
                    nc.tensor.matmul(
                        ps[:rp, :MCH], wqT[:, kt, 128 * rc:128 * rc + rp],
                        hsT[:, kt, :], start=(kt == 0), stop=(kt == KT - 1))
                    if kt % 12 == 11:
                        yield
                nc.vector.tensor_copy(
                    fusedT[:rp, rc, MCH * mc:MCH * (mc + 1)], ps[:rp, :MCH])
                yield

        def rope_units(n, hfs=(0, 1)):
            """Generator: RoPE in place on q rows and the k row of batch n."""
            for hf in hfs:
                sl = slice(512 * hf, 512 * (hf + 1))
                for rc in range(5):
                    x = fusedT[:, rc, L * n:L * (n + 1)]
                    pp = psM.tile([128, 512], F32, tag="misc")
                    nc.tensor.matmul(pp[:], prope[:], x[:, sl],
                                     start=True, stop=True)
                    a = workp.tile([128, 512], BF16, tag="ropea")
                    b = workp.tile([128, 512], BF16, tag="ropeb")
                    nc.vector.tensor_mul(a[:], x[:, sl], cosT[:, sl])
                    nc.vector.tensor_mul(b[:], pp[:], sinT[:, sl])
                    nc.vector.tensor_add(x[:, sl], a[:], b[:])
                    yield

        def prep_vt(n, jts):
            """Generator: v transposes for batch n, given 128-key blocks."""
            for jt in jts:
                tp = psM.tile([128, 1024], BF16, tag="misc")
                nc.tensor.transpose(
                    tp[:, 0:64], fusedT[0:64, 5, L * n + 128 * jt:L * n + 128 * (jt + 1)],
                    id64[:])
                nc.vector.tensor_copy(v_nat[:, 8 * n + jt, 0:DKV], tp[:, 0:64])
                yield

        def prep_kdup(n):
            """Generator: kT duplication into both partition halves so
            lhsT/rhs base partitions match for every head parity."""
            nc.gpsimd.dma_start(kT_dup[0:64, n, :], fusedT[64:128, 4, L * n:L * (n + 1)])
            nc.gpsimd.dma_start(kT_dup[64:128, n, :], fusedT[64:128, 4, L * n:L * (n + 1)])
            yield

        def attn_head(n, h):
            """Generator: one attention head, yielding between j-tile units."""
            poff = (64 * h) % 128
            prc = (64 * h) // 128
            kTn = kT_dup[poff:poff + 64, n, :]
            qh = fusedT[poff:poff + 64, prc, L * n:L * (n + 1)]
            for qc in range(2):
                av = psAV.tile([128, 512], F32, tag="av")
                njt = 4 * (qc + 1)
                pend = None
                for jt in range(njt):
                    off = max(0, 128 * jt - 512 * qc)
                    sp = psS.tile([128, 512], F32, tag="sp")
                    nc.tensor.matmul(
                        sp[:, 0:512 - off],
                        kTn[:, 128 * jt:128 * (jt + 1)],
                        qh[:, 512 * qc + off:512 * (qc + 1)],
                        start=True, stop=True)
                    et = expp.tile([128, 512], BF16, tag="exp")
                    nc.scalar.activation(
                        et[:, off:512], sp[:, 0:512 - off],
                        mybir.ActivationFunctionType.Exp,
                        scale=1.0 / math.sqrt(DKV))
                    if 128 * jt >= 512 * qc:
                        nc.vector.tensor_mul(
                            et[:, off:off + 128], et[:, off:off + 128], tri[:])
                    if pend is not None:
                        pjt, po, pet = pend
                        nc.tensor.matmul(
                            av[:, po:512], v_nat[:, 8 * n + pjt, :], pet[:, po:512],
                            start=(pjt == 0), stop=False)
                    pend = (jt, off, et)
                    yield
                pjt, po, pet = pend
                nc.tensor.matmul(
                    av[:, po:512], v_nat[:, 8 * n + pjt, :], pet[:, po:512],
                    start=(pjt == 0), stop=True)
                # reciprocal_approx_fast mis-executes on base-shifted inputs
                # (HW-verified), so realign the denominator rows to partition 0
                # with a copy first.
                den = workp.tile([64, 512], F32, tag="den")
                nc.vector.tensor_copy(den[:], av[64:128, :])
                rec = workp.tile([64, 512], F32, tag="rec")
                nc.vector.reciprocal_approx_fast(rec[:], den[:])
                yield
                nc.vector.tensor_mul(
                    attn_sb[poff:poff + 64, prc, L * n + 512 * qc:L * n + 512 * (qc + 1)],
                    av[0:64, :], rec[:])
                yield

        def drive(n, fillers, prime=0, ratio=1.0):
            """Drive all heads of batch n with 2 rolling in-flight generators,
            advancing ~ratio filler yields per attention step."""

            def advance():
                if fillers:
                    try:
                        next(fillers[0])
                    except StopIteration:
                        fillers.popleft()

            for _ in range(prime):
                advance()
            nxt = 0
            slots = []

            def spawn():
                nonlocal nxt
                if nxt < HPC:
                    slots.append(attn_head(n, nxt))
                    nxt += 1

            spawn()
            spawn()
            step = 0
            credit = 0.0
            while slots:
                for g in list(slots):
                    try:
                        next(g)
                    except StopIteration:
                        slots.remove(g)
                        spawn()
                step += 1
                credit += ratio
                while credit >= 1.0:
                    credit -= 1.0
                    advance()

        def drain(fillers):
            while fillers:
                f = fillers.popleft()
                for _ in f:
                    pass

        def qkv_chunk0_ktouter():
            """First token chunk with the kt loop outermost: consumes wqT
            kt-tiles at DMA-arrival rate so the cold-start stall shrinks.
            Uses one accumulator bank from each PSUM pool (nothing else is
            live yet)."""
            accs = [psQ.tile([128, 512], F32, tag="acc", name="acc0"),
                    psQ.tile([128, 512], F32, tag="acc", name="acc1"),
                    psS.tile([128, 512], F32, tag="sp", name="acc2"),
                    psS.tile([128, 512], F32, tag="sp", name="acc3"),
                    psAV.tile([128, 512], F32, tag="av", name="acc4"),
                    psAV.tile([128, 512], F32, tag="av", name="acc5")]
            for kt in range(KT):
                for rc in range(RC):
                    rp = 128 if rc < 5 else 64
                    nc.tensor.matmul(
                        accs[rc][:rp, :MCH], wqT[:, kt, 128 * rc:128 * rc + rp],
                        hsT0[:, kt, :], start=(kt == 0), stop=(kt == KT - 1))
            for rc in range(RC):
                rp = 128 if rc < 5 else 64
                nc.vector.tensor_copy(fusedT[:rp, rc, 0:MCH], accs[rc][:rp, :MCH])

        # ---- window 1: QKV batch 0 ----
        qkv_chunk0_ktouter()
        for mc in range(1, NMC // 2):
            for _ in qkv_chunk(mc):
                pass
        # rope(b0) interleaved with the first QKV(b1) chunk to avoid a PE gap
        w1fill = deque([qkv_chunk(4)])
        for g in (rope_units(0), prep_vt(0, range(8)), prep_kdup(0)):
            for _ in g:
                if w1fill:
                    try:
                        next(w1fill[0])
                    except StopIteration:
                        w1fill.popleft()

        # stage B (dense weights + output staging) opens mid-drive, as soon
        # as the last QKV(b1) filler has been emitted and wqT/hsT are free —
        # so wdT2 is resident the moment attention(b0) finishes.
        stageB = ExitStack()
        dref = {}

        def stage_switch():
            stageA.close()
            wdp = stageB.enter_context(tc.tile_pool(name="wd", bufs=1))
            otp = stageB.enter_context(tc.tile_pool(name="ot", bufs=2))
            wdT2 = wdp.tile([128, QPAD // 128, D], BF16)
            wd_r = wd_bf[:].rearrange("(kt p) c -> p kt c", p=128)
            for kt, eng in enumerate((nc.gpsimd, nc.scalar, nc.sync,
                                      nc.gpsimd, nc.scalar)):
                eng.dma_start(wdT2[:, kt, :], wd_r[:, kt, :])
            dref["wdT2"] = wdT2
            dref["otp"] = otp
            yield

        # ---- window 2: attention(b0) || QKV(b1) + rope(b1) + prep(b1) ----
        # rope/prep(b1) slices follow the QKV(b1) chunks they depend on, so
        # attention(b1) is unblocked as early as possible.
        fillers1 = deque(w1fill)
        fillers1.append(qkv_chunk(5))
        fillers1.append(rope_units(1, hfs=(0,)))
        fillers1.append(prep_vt(1, range(0, 4)))
        fillers1.append(qkv_chunk(6))
        fillers1.append(qkv_chunk(7))
        fillers1.append(stage_switch())
        fillers1.append(rope_units(1, hfs=(1,)))
        fillers1.append(prep_vt(1, range(4, 8)))
        fillers1.append(prep_kdup(1))
        drive(0, fillers1)
        drain(fillers1)

        CCH = [512] * 8 + [448]          # dense column chunks (sum = 4544)

        def dense_mtile(n, mt, stream_out=False):
            """Generator: partial dense for one m-tile; one big out DMA
            (or per-chunk DMAs when stream_out, to shrink the kernel tail)."""
            wdT2 = dref["wdT2"]
            ot = dref["otp"].tile([128, D], BF16, tag="ot")
            rows = slice(L * n + 128 * mt, L * n + 128 * (mt + 1))
            col = 0
            for w in CCH:
                pa = psQ.tile([128, 512], F32, tag="acc")
                for kt in range(QPAD // 128):
                    nc.tensor.matmul(
                        pa[:, :w], attn_sb[:, kt, rows],
                        wdT2[:, kt, col:col + w],
                        start=(kt == 0), stop=(kt == QPAD // 128 - 1))
                nc.vector.tensor_copy(ot[:, col:col + w], pa[:, :w])
                if stream_out:
                    nc.gpsimd.dma_start(out[rows, col:col + w], ot[:, col:col + w])
                col += w
                yield
            if not stream_out:
                nc.gpsimd.dma_start(out[rows, :], ot[:])

        # ---- window 3: attention(b1) || dense(b0) ----
        fillers2 = deque(dense_mtile(0, mt) for mt in range(8))
        drive(1, fillers2)
        drain(fillers2)

        # ---- window 4: dense(b1) ----
        drain(deque(dense_mtile(1, mt) for mt in range(8)))
        stageB.close()

    nc.compile()
    return nc


_NC_CACHE = None


def _get_nc():
    global _NC_CACHE
    if _NC_CACHE is None:
        _NC_CACHE = _build()
    return _NC_CACHE


def _host_inputs(hidden_states, w_qkv, w_dense):
    """Build the per-core input maps (transpose + slice + bf16 cast on host)."""
    hs = np.asarray(hidden_states, dtype=np.float32).reshape(M, D)
    w_qkv = np.asarray(w_qkv, dtype=np.float32)
    w_dense = np.asarray(w_dense, dtype=np.float32)
    hs_bf = np.zeros((DP, M), dtype=ml_dtypes.bfloat16)
    hs_bf[:D, :] = np.ascontiguousarray(hs.T).astype(ml_dtypes.bfloat16)

    # RoPE tables, transposed to [dkv, l], duplicated on partitions 0-63 / 64-127
    inv_freq = 1.0 / (ROPE_BASE ** (np.arange(0, DKV, 2, dtype=np.float32) / DKV))
    t = np.arange(L, dtype=np.float32)
    freqs = np.outer(t, inv_freq)
    emb = np.concatenate([freqs, freqs], axis=-1)        # [L, DKV]
    cosT = np.cos(emb).T.astype(np.float32)              # [DKV, L]
    sinT = np.sin(emb).T.astype(np.float32)
    cos2 = np.concatenate([cosT, cosT], axis=0).astype(ml_dtypes.bfloat16)
    sin2 = np.concatenate([sinT, sinT], axis=0).astype(ml_dtypes.bfloat16)

    # tri[j, q] = 1 if j <= q (within-tile causal mask)
    tri = (np.arange(128)[:, None] <= np.arange(128)[None, :]).astype(
        ml_dtypes.bfloat16)

    # RoPE rotation: (P x)[d] = -x[d+32] (d<32), x[d-32] (d>=32); lhsT = P.T, 2 blocks
    P1 = np.zeros((DKV, DKV), dtype=np.float32)
    for d in range(32):
        P1[d, d + 32] = -1.0
        P1[d + 32, d] = 1.0
    PT = P1.T
    prope2 = np.zeros((128, 128), dtype=np.float32)
    prope2[:64, :64] = PT
    prope2[64:, 64:] = PT
    prope2 = prope2.astype(ml_dtypes.bfloat16)

    ident64 = np.eye(64, dtype=np.float32).astype(ml_dtypes.bfloat16)

    kv_bf = w_qkv[H * DKV:, :].T.astype(ml_dtypes.bfloat16)   # [D, 128]
    in_maps = []
    for c in range(NCORES):
        h0 = HPC * c
        nh = min(HPC, H - h0)
        wq_loc = np.zeros((DP, RROWS), dtype=ml_dtypes.bfloat16)
        wq_loc[:D, :nh * DKV] = w_qkv[h0 * DKV:(h0 + nh) * DKV, :].T.astype(
            ml_dtypes.bfloat16)
        wq_loc[:D, QROWS:] = kv_bf

        # dense weight rows for this core's heads: w_dense columns
        # [64*h0 : 64*(h0+nh)) transposed, zero-padded to QPAD rows
        wd_loc = np.zeros((QPAD, D), dtype=ml_dtypes.bfloat16)
        wd_loc[:nh * DKV, :] = w_dense[:, DKV * h0:DKV * (h0 + nh)].T.astype(
            ml_dtypes.bfloat16)

        in_maps.append({
            "hs_bf": hs_bf,
            "wq_bf": wq_loc,
            "wd_bf": wd_loc,
            "cos2": cos2,
            "sin2": sin2,
            "tri": tri,
            "prope2": prope2,
            "ident64": ident64,
            "colones": np.ones((128, 1024), dtype=ml_dtypes.bfloat16),
        })
    return in_maps


def kernel(hidden_states, w_qkv, w_dense, _trace=False, _trace_kwargs=None):
    nc = _get_nc()
    in_maps = _host_inputs(hidden_states, w_qkv, w_dense)
    kw = {}
    if _trace:
        kw = dict(trace=True, **(_trace_kwargs or {}))
    res = run_bass_kernel_spmd(nc, in_maps, list(range(NCORES)), **kw)
    full = res.results[0]["out"].astype(np.float32)
    for c in range(1, NCORES):
        full += res.results[c]["out"].astype(np.float32)
    kernel._last_exec_time_ns = res.exec_time_ns
    return full.reshape(N, L, D).astype(np.float32)


# revision 36
# speedup vs baseline: 1.1933x; 1.0151x over previous
"""Falcon-style MQA attention (71 heads, 1 KV head, RoPE, causal) on 8 TRN2 NeuronCores.

Sharding: tensor-parallel over query heads (9 per core, core 7 has 8 + a zero-pad
head), the single KV head replicated. Per core: QKV projection for its heads
(+KV), RoPE, causal flash-style attention in transposed layout, then a PARTIAL
dense projection over the core's own head rows for all 4544 output columns.
The host sums the 8 partial outputs (contraction-sharded dense = host reduce);
no device collective is needed. All operand transposes are done host-side.

Schedule (v2): the kernel is software-pipelined across the two batches so the
tensor engine always has dense matmul work (keeps the PE HAM clock warm):
  QKV(b0) -> [attn(b0) || QKV(b1)+RoPE(b1)+prep(b1)] -> [attn(b1) || dense(b0)]
  -> dense(b1)
Interleaving is done at emission time via generators (the Tile scheduler
executes per-engine streams roughly in program order).

Self-contained: hardcodes all shapes; needs only numpy + ml_dtypes + concourse.
"""

import math
from collections import deque
from contextlib import ExitStack

import numpy as np
import ml_dtypes

import concourse.bass as bass
import concourse.mybir as mybir
import concourse.tile as tile
from concourse import bacc
from concourse.bass_utils import run_bass_kernel_spmd

NCORES = 8
N, L, D = 2, 1024, 4544
H, DKV = 71, 64
M = N * L                    # 2048 tokens
DP = 4608                    # D padded to 36*128
KT = DP // 128               # 36 contraction tiles for QKV
HPC = 9                      # head slots per core (core 7: 8 real + 1 zero-pad)
QROWS = HPC * DKV            # 576 attention rows per core
QPAD = 640                   # padded to 5*128 for the dense contraction
RROWS = QROWS + 2 * DKV      # 704 fused rows per core (q + k + v)
RC = 6                       # row-chunks of fusedT (5x128 + 64)
MCH = 256                    # QKV token-chunk width
NMC = M // MCH               # 8 token chunks (0-3 batch 0, 4-7 batch 1)
ROPE_BASE = 10000.0

F32 = mybir.dt.float32
F32R = mybir.dt.float32r
BF16 = mybir.dt.bfloat16


def _build():
    nc = bacc.Bacc("TRN2", target_bir_lowering=False, debug=False, num_devices=NCORES)

    hs_bf = nc.dram_tensor("hs_bf", [DP, M], BF16, kind="ExternalInput")      # hs.T
    wq_bf = nc.dram_tensor("wq_bf", [DP, RROWS], BF16, kind="ExternalInput")  # wq_loc.T
    wd_bf = nc.dram_tensor("wd_bf", [QPAD, D], BF16, kind="ExternalInput")    # wd rows for local heads
    cos2 = nc.dram_tensor("cos2", [128, L], BF16, kind="ExternalInput")
    sin2 = nc.dram_tensor("sin2", [128, L], BF16, kind="ExternalInput")
    tri_in = nc.dram_tensor("tri", [128, 128], BF16, kind="ExternalInput")
    prope2 = nc.dram_tensor("prope2", [128, 128], BF16, kind="ExternalInput")
    ident64 = nc.dram_tensor("ident64", [64, 64], BF16, kind="ExternalInput")
    colones = nc.dram_tensor("colones", [128, 1024], BF16, kind="ExternalInput")
    out = nc.dram_tensor("out", [M, D], BF16, kind="ExternalOutput")

    with tile.TileContext(nc) as tc, ExitStack() as top:
        constp = top.enter_context(tc.tile_pool(name="const", bufs=1))
        workp = top.enter_context(tc.tile_pool(name="work", bufs=3))
        expp = top.enter_context(tc.tile_pool(name="exps", bufs=6))
        psQ = top.enter_context(tc.tile_pool(name="psQ", bufs=2, space="PSUM"))
        psS = top.enter_context(tc.tile_pool(name="psS", bufs=3, space="PSUM"))
        psAV = top.enter_context(tc.tile_pool(name="psAV", bufs=2, space="PSUM"))
        psM = top.enter_context(tc.tile_pool(name="psM", bufs=1, space="PSUM"))

        fusedp = top.enter_context(tc.tile_pool(name="fused", bufs=1))
        fusedT = fusedp.tile([128, RC, M], BF16)

        attnp = top.enter_context(tc.tile_pool(name="attn", bufs=1))
        attn_sb = attnp.tile([128, QPAD // 128, M], BF16)
        kT_dup = attnp.tile([128, N, L], BF16)
        # v_nat columns 0-63: v.T per key block; columns 64-127: all-ones, so
        # the AV matmul's output rows 64-127 are 64 broadcast copies of the
        # softmax denominator (enables a 64-wide parallel reciprocal).
        v_nat = attnp.tile([128, N * 8, 2 * DKV], BF16)

        stageA = ExitStack()
        wqp = stageA.enter_context(tc.tile_pool(name="wq", bufs=1))
        hstp = stageA.enter_context(tc.tile_pool(name="hst", bufs=3))

        hs_r = hs_bf[:].rearrange("(kt p) m -> p kt m", p=128)

        # Startup: hs chunk 0 alone on the sync queue (the kt-outer sweep
        # needs it promptly); wqT split across the gpsimd+scalar queues.
        hsT0 = hstp.tile([128, KT, MCH], BF16, tag="hsT")
        nc.sync.dma_start(hsT0[:, 0:12, :], hs_r[:, 0:12, 0:MCH])
        wqT = wqp.tile([128, KT, RROWS], BF16)
        wq_r = wq_bf[:].rearrange("(kt p) r -> p kt r", p=128)
        for kg, eng in enumerate((nc.gpsimd, nc.scalar, nc.gpsimd,
                                  nc.scalar, nc.gpsimd, nc.scalar)):
            eng.dma_start(wqT[:, 6 * kg:6 * (kg + 1), :],
                          wq_r[:, 6 * kg:6 * (kg + 1), :])
        nc.sync.dma_start(hsT0[:, 12:KT, :], hs_r[:, 12:KT, 0:MCH])

        # ---- constants (after the hot startup DMAs) ----
        cosT = constp.tile([128, L], BF16)
        sinT = constp.tile([128, L], BF16)
        tri = constp.tile([128, 128], BF16)
        prope = constp.tile([128, 128], BF16)
        id64 = constp.tile([64, 64], BF16)
        nc.gpsimd.dma_start(cosT[:], cos2[:])
        nc.gpsimd.dma_start(sinT[:], sin2[:])
        nc.gpsimd.dma_start(tri[:], tri_in[:])
        nc.gpsimd.dma_start(prope[:], prope2[:])
        nc.gpsimd.dma_start(id64[:], ident64[:])
        nc.vector.memset(attn_sb[64:128, 4, :], 0.0)
        nc.gpsimd.dma_start(v_nat[:, :, DKV:2 * DKV],
                            colones[:].rearrange("p (s o) -> p s o", o=DKV))

        def qkv_chunk(mc, hsT=None):
            """Generator: QKV projection for one token chunk of MCH tokens."""
            if hsT is None:
                hsT = hstp.tile([128, KT, MCH], BF16, tag="hsT")
                nc.sync.dma_start(hsT[:], hs_r[:, :, MCH * mc:MCH * (mc + 1)])
            yield
            for rc in range(RC):
                rp = 128 if rc < 5 else 64
                ps = psQ.tile([128, 512], F32, tag="acc")
                for kt in range(KT):
                    nc.tensor.matmul(
                        ps[:rp, :MCH], wqT[:, kt, 128 * rc:128 * rc + rp],
                        hsT[:, kt, :], start=(kt == 0), stop=(kt == KT - 1))
                    if kt % 12 == 11:
                        yield
                nc.vector.tensor_copy(
                    fusedT[:rp, rc, MCH * mc:MCH * (mc + 1)], ps[:rp, :MCH])
                yield

        def rope_units(n, hfs=(0, 1)):
            """Generator: RoPE in place on q rows and the k row of batch n."""
            for hf in hfs:
                sl = slice(512 * hf, 512 * (hf + 1))
                for rc in range(5):
                    x = fusedT[:, rc, L * n:L * (n + 1)]
                    pp = psM.tile([128, 512], F32, tag="misc")
                    nc.tensor.matmul(pp[:], prope[:], x[:, sl],
                                     start=True, stop=True)
                    a = workp.tile([128, 512], BF16, tag="ropea")
                    b = workp.tile([128, 512], BF16, tag="ropeb")
                    nc.vector.tensor_mul(a[:], x[:, sl], cosT[:, sl])
                    nc.vector.tensor_mul(b[:], pp[:], sinT[:, sl])
                    nc.vector.tensor_add(x[:, sl], a[:], b[:])
                    yield

        def prep_vt(n, jts):
            """Generator: v transposes for batch n, given 128-key blocks."""
            for jt in jts:
                tp = psM.tile([128, 1024], BF16, tag="misc")
                nc.tensor.transpose(
                    tp[:, 0:64], fusedT[0:64, 5, L * n + 128 * jt:L * n + 128 * (jt + 1)],
                    id64[:])
                nc.vector.tensor_copy(v_nat[:, 8 * n + jt, 0:DKV], tp[:, 0:64])
                yield

        def prep_kdup(n):
            """Generator: kT duplication into both partition halves so
            lhsT/rhs base partitions match for every head parity."""
            nc.gpsimd.dma_start(kT_dup[0:64, n, :], fusedT[64:128, 4, L * n:L * (n + 1)])
            nc.gpsimd.dma_start(kT_dup[64:128, n, :], fusedT[64:128, 4, L * n:L * (n + 1)])
            yield

        def attn_head(n, h):
            """Generator: one attention head, yielding between j-tile units."""
            poff = (64 * h) % 128
            prc = (64 * h) // 128
            kTn = kT_dup[poff:poff + 64, n, :]
            qh = fusedT[poff:poff + 64, prc, L * n:L * (n + 1)]
            for qc in range(2):
                av = psAV.tile([128, 512], F32, tag="av")
                njt = 4 * (qc + 1)
                pend = None
                for jt in range(njt):
                    off = max(0, 128 * jt - 512 * qc)
                    sp = psS.tile([128, 512], F32, tag="sp")
                    nc.tensor.matmul(
                        sp[:, 0:512 - off],
                        kTn[:, 128 * jt:128 * (jt + 1)],
                        qh[:, 512 * qc + off:512 * (qc + 1)],
                        start=True, stop=True)
                    et = expp.tile([128, 512], BF16, tag="exp")
                    nc.scalar.activation(
                        et[:, off:512], sp[:, 0:512 - off],
                        mybir.ActivationFunctionType.Exp,
                        scale=1.0 / math.sqrt(DKV))
                    if 128 * jt >= 512 * qc:
                        nc.vector.tensor_mul(
                            et[:, off:off + 128], et[:, off:off + 128], tri[:])
                    if pend is not None:
                        pjt, po, pet = pend
                        nc.tensor.matmul(
                            av[:, po:512], v_nat[:, 8 * n + pjt, :], pet[:, po:512],
                            start=(pjt == 0), stop=False)
                    pend = (jt, off, et)
                    yield
                pjt, po, pet = pend
                nc.tensor.matmul(
                    av[:, po:512], v_nat[:, 8 * n + pjt, :], pet[:, po:512],
                    start=(pjt == 0), stop=True)
                # reciprocal_approx_fast mis-executes on base-shifted inputs
                # (HW-verified), so realign the denominator rows to partition 0
                # with a copy first.
                den = workp.tile([64, 512], F32, tag="den")
                nc.vector.tensor_copy(den[:], av[64:128, :])
                rec = workp.tile([64, 512], F32, tag="rec")
                nc.vector.reciprocal_approx_fast(rec[:], den[:])
                yield
                nc.vector.tensor_mul(
                    attn_sb[poff:poff + 64, prc, L * n + 512 * qc:L * n + 512 * (qc + 1)],
                    av[0:64, :], rec[:])
                yield

        def drive(n, fillers, prime=0, ratio=1.0):
            """Drive all heads of batch n with 2 rolling in-flight generators,
            advancing ~ratio filler yields per attention step."""

            def advance():
                if fillers:
                    try:
                        next(fillers[0])
                    except StopIteration:
                        fillers.popleft()

            for _ in range(prime):
                advance()
            nxt = 0
            slots = []

            def spawn():
                nonlocal nxt
                if nxt < HPC:
                    slots.append(attn_head(n, nxt))
                    nxt += 1

            spawn()
            spawn()
            step = 0
            credit = 0.0
            while slots:
                for g in list(slots):
                    try:
                        next(g)
                    except StopIteration:
                        slots.remove(g)
                        spawn()
                step += 1
                credit += ratio
                while credit >= 1.0:
                    credit -= 1.0
                    advance()

        def drain(fillers):
            while fillers:
                f = fillers.popleft()
                for _ in f:
                    pass

        def qkv_chunk0_ktouter():
            """First token chunk with the kt loop outermost: consumes wqT
            kt-tiles at DMA-arrival rate so the cold-start stall shrinks.
            Uses one accumulator bank from each PSUM pool (nothing else is
            live yet)."""
            accs = [psQ.tile([128, 512], F32, tag="acc", name="acc0"),
                    psQ.tile([128, 512], F32, tag="acc", name="acc1"),
                    psS.tile([128, 512], F32, tag="sp", name="acc2"),
                    psS.tile([128, 512], F32, tag="sp", name="acc3"),
                    psAV.tile([128, 512], F32, tag="av", name="acc4"),
                    psAV.tile([128, 512], F32, tag="av", name="acc5")]
            for kt in range(KT):
                for rc in range(RC):
                    rp = 128 if rc < 5 else 64
                    nc.tensor.matmul(
                        accs[rc][:rp, :MCH], wqT[:, kt, 128 * rc:128 * rc + rp],
                        hsT0[:, kt, :], start=(kt == 0), stop=(kt == KT - 1))
            for rc in range(RC):
                rp = 128 if rc < 5 else 64
                nc.vector.tensor_copy(fusedT[:rp, rc, 0:MCH], accs[rc][:rp, :MCH])

        # ---- window 1: QKV batch 0 ----
        qkv_chunk0_ktouter()
        for mc in range(1, NMC // 2):
            for _ in qkv_chunk(mc):
                pass
        # rope(b0) interleaved with the first QKV(b1) chunk to avoid a PE gap
        def qkv_pair(mc_a, mc_b, hf, vt_jts):
            """Generator: two QKV(b1) chunks interleaved rc-wise with the
            rope/kdup/v-transpose units that become ready per rc — so batch-1
            prep dissolves into the filler stream instead of trailing it."""
            ga = qkv_chunk(mc_a)
            gb = qkv_chunk(mc_b)
            next(ga)
            next(gb)
            yield
            rope = rope_units(1, hfs=(hf,))
            for rc in range(RC):
                for g in (ga, gb):
                    for _ in range(4):
                        next(g)
                        yield
                if rc < 5:
                    next(rope)
                    yield
                if rc == 4 and hf == 1:
                    for _ in prep_kdup(1):
                        yield
                if rc == 5:
                    for _ in prep_vt(1, vt_jts):
                        yield

        w1fill = deque([qkv_pair(4, 5, 0, range(0, 4))])
        for g in (rope_units(0), prep_vt(0, range(8)), prep_kdup(0)):
            for _ in g:
                if w1fill:
                    try:
                        next(w1fill[0])
                    except StopIteration:
                        w1fill.popleft()

        # stage B (dense weights + output staging) opens mid-drive, as soon
        # as the last QKV(b1) filler has been emitted and wqT/hsT are free —
        # so wdT2 is resident the moment attention(b0) finishes.
        stageB = ExitStack()
        dref = {}

        def stage_switch():
            stageA.close()
            wdp = stageB.enter_context(tc.tile_pool(name="wd", bufs=1))
            otp = stageB.enter_context(tc.tile_pool(name="ot", bufs=2))
            wdT2 = wdp.tile([128, QPAD // 128, D], BF16)
            wd_r = wd_bf[:].rearrange("(kt p) c -> p kt c", p=128)
            for kt, eng in enumerate((nc.gpsimd, nc.scalar, nc.sync,
                                      nc.gpsimd, nc.scalar)):
                eng.dma_start(wdT2[:, kt, :], wd_r[:, kt, :])
            dref["wdT2"] = wdT2
            dref["otp"] = otp
            yield

        # ---- window 2: attention(b0) || QKV(b1) + rope(b1) + prep(b1) ----
        # rope/prep(b1) slices follow the QKV(b1) chunks they depend on, so
        # attention(b1) is unblocked as early as possible.
        fillers1 = deque(w1fill)
        fillers1.append(qkv_pair(6, 7, 1, range(4, 8)))
        fillers1.append(stage_switch())
        drive(0, fillers1, ratio=1.6)
        drain(fillers1)

        CCH = [512] * 8 + [448]          # dense column chunks (sum = 4544)

        def dense_mtile(n, mt, stream_out=False):
            """Generator: partial dense for one m-tile; one big out DMA
            (or per-chunk DMAs when stream_out, to shrink the kernel tail)."""
            wdT2 = dref["wdT2"]
            ot = dref["otp"].tile([128, D], BF16, tag="ot")
            rows = slice(L * n + 128 * mt, L * n + 128 * (mt + 1))
            col = 0
            for w in CCH:
                pa = psQ.tile([128, 512], F32, tag="acc")
                for kt in range(QPAD // 128):
                    nc.tensor.matmul(
                        pa[:, :w], attn_sb[:, kt, rows],
                        wdT2[:, kt, col:col + w],
                        start=(kt == 0), stop=(kt == QPAD // 128 - 1))
                nc.vector.tensor_copy(ot[:, col:col + w], pa[:, :w])
                if stream_out:
                    nc.gpsimd.dma_start(out[rows, col:col + w], ot[:, col:col + w])
                col += w
                yield
            if not stream_out:
                nc.gpsimd.dma_start(out[rows, :], ot[:])

        # ---- window 3: attention(b1) || dense(b0) ----
        fillers2 = deque(dense_mtile(0, mt) for mt in range(8))
        drive(1, fillers2, ratio=1.2)
        drain(fillers2)

        # ---- window 4: dense(b1) ----
        drain(deque(dense_mtile(1, mt, stream_out=(mt == 7))
                    for mt in range(8)))
        stageB.close()

    nc.compile()
    return nc


_NC_CACHE = None


def _get_nc():
    global _NC_CACHE
    if _NC_CACHE is None:
        _NC_CACHE = _build()
    return _NC_CACHE


def _host_inputs(hidden_states, w_qkv, w_dense):
    """Build the per-core input maps (transpose + slice + bf16 cast on host)."""
    hs = np.asarray(hidden_states, dtype=np.float32).reshape(M, D)
    w_qkv = np.asarray(w_qkv, dtype=np.float32)
    w_dense = np.asarray(w_dense, dtype=np.float32)
    hs_bf = np.zeros((DP, M), dtype=ml_dtypes.bfloat16)
    hs_bf[:D, :] = np.ascontiguousarray(hs.T).astype(ml_dtypes.bfloat16)

    # RoPE tables, transposed to [dkv, l], duplicated on partitions 0-63 / 64-127
    inv_freq = 1.0 / (ROPE_BASE ** (np.arange(0, DKV, 2, dtype=np.float32) / DKV))
    t = np.arange(L, dtype=np.float32)
    freqs = np.outer(t, inv_freq)
    emb = np.concatenate([freqs, freqs], axis=-1)        # [L, DKV]
    cosT = np.cos(emb).T.astype(np.float32)              # [DKV, L]
    sinT = np.sin(emb).T.astype(np.float32)
    cos2 = np.concatenate([cosT, cosT], axis=0).astype(ml_dtypes.bfloat16)
    sin2 = np.concatenate([sinT, sinT], axis=0).astype(ml_dtypes.bfloat16)

    # tri[j, q] = 1 if j <= q (within-tile causal mask)
    tri = (np.arange(128)[:, None] <= np.arange(128)[None, :]).astype(
        ml_dtypes.bfloat16)

    # RoPE rotation: (P x)[d] = -x[d+32] (d<32), x[d-32] (d>=32); lhsT = P.T, 2 blocks
    P1 = np.zeros((DKV, DKV), dtype=np.float32)
    for d in range(32):
        P1[d, d + 32] = -1.0
        P1[d + 32, d] = 1.0
    PT = P1.T
    prope2 = np.zeros((128, 128), dtype=np.float32)
    prope2[:64, :64] = PT
    prope2[64:, 64:] = PT
    prope2 = prope2.astype(ml_dtypes.bfloat16)

    ident64 = np.eye(64, dtype=np.float32).astype(ml_dtypes.bfloat16)

    kv_bf = w_qkv[H * DKV:, :].T.astype(ml_dtypes.bfloat16)   # [D, 128]
    in_maps = []
    for c in range(NCORES):
        h0 = HPC * c
        nh = min(HPC, H - h0)
        wq_loc = np.zeros((DP, RROWS), dtype=ml_dtypes.bfloat16)
        wq_loc[:D, :nh * DKV] = w_qkv[h0 * DKV:(h0 + nh) * DKV, :].T.astype(
            ml_dtypes.bfloat16)
        wq_loc[:D, QROWS:] = kv_bf

        # dense weight rows for this core's heads: w_dense columns
        # [64*h0 : 64*(h0+nh)) transposed, zero-padded to QPAD rows
        wd_loc = np.zeros((QPAD, D), dtype=ml_dtypes.bfloat16)
        wd_loc[:nh * DKV, :] = w_dense[:, DKV * h0:DKV * (h0 + nh)].T.astype(
            ml_dtypes.bfloat16)

        in_maps.append({
            "hs_bf": hs_bf,
            "wq_bf": wq_loc,
            "wd_bf": wd_loc,
            "cos2": cos2,
            "sin2": sin2,
            "tri": tri,
            "prope2": prope2,
            "ident64": ident64,
            "colones": np.ones((128, 1024), dtype=ml_dtypes.bfloat16),
        })
    return in_maps


def kernel(hidden_states, w_qkv, w_dense, _trace=False, _trace_kwargs=None):
    nc = _get_nc()
    in_maps = _host_inputs(hidden_states, w_qkv, w_dense)
    kw = {}
    if _trace:
        kw = dict(trace=True, **(_trace_kwargs or {}))
    res = run_bass_kernel_spmd(nc, in_maps, list(range(NCORES)), **kw)
    full = res.results[0]["out"].astype(np.float32)
    for c in range(1, NCORES):
        full += res.results[c]["out"].astype(np.float32)
    kernel._last_exec_time_ns = res.exec_time_ns
    return full.reshape(N, L, D).astype(np.float32)
